# revision 30
# baseline (speedup 1.0000x reference)
"""Trainium2 Bass kernel for a 2-layer GAT (nn_GAT_197568496078).

Strategy (8 NeuronCores, SPMD single program):
  - Edges (+self loops) are sharded by DESTINATION node range: core c owns
    dst in [c*6250, (c+1)*6250). Aggregation is then core-local (no
    collectives). The node feature table is replicated (each core builds it
    with small matmuls).
  - Per layer, each core builds an HBM table T1[n] = [h(128) | alpha_src(2)]
    (fp16, 512B rows) and a small per-shard table T2[v_local] = alpha_dst(2),
    then streams its edges one dst-WINDOW (127 dst nodes) at a time:
      gather T1 rows by src (dma_gather, two instructions: src < 25024 and
      src >= 25024, so indices fit int16),
      one-hot S[e, j] = (iota_j == dst_rel_e)  (bf16, DVE),
      alpha_dst per edge WITHOUT a per-edge gather: AdRep[p, h, j] =
        alpha_dst of the window's 127 dst nodes replicated to all partitions
        (rank-1 ones-matmul broadcast), then
        ad[e, t, h] = reduce_j S[e,t,j] * AdRep[e,h,j]   (DVE mult+reduce),
      score = as + ad; leaky-relu = max(x, .2x); w = exp(score)  (bf16),
      M = [h * w | w]  (bf16),
      psum[j, 0:130] += S.T @ M   (TensorE, fp32 PSUM, per 127-dst window)
    flush: out[j] = msg/denom (+bias, relu/mean-heads).
  - Eliminating the per-edge alpha_dst gather halves the GpSimd descriptor
    generation work, which is the bottleneck engine (dma_gather runs on 2 of
    the 8 Q7 cores and serializes on the GpSimd engine at ~6ns/row).
  - Softmax is computed without the segment-max shift: exp() in bf16 has
    the range for scores in [-16, 16]; the max-shift cancels exactly in the
    reference so results match to ~1e-3.
  - Two launches (layer 1, layer 2); the host re-shards layer-1 output
    between them (index-only work).
  - Per-window tile counts are padded to the max over cores so all 8 cores
    run one identical program; all per-core variation lives in data arrays
    (gather indices, dst_rel).
"""
import os
import sys
import numpy as np
import ml_dtypes

sys.path.insert(0, "/opt/trn_rl_repo")

import concourse.bacc as bacc   # noqa: E402
import concourse.bass as bass   # noqa: E402
import concourse.mybir as mybir # noqa: E402
import concourse.tile as tile   # noqa: E402
from concourse.alu_op_type import AluOpType          # noqa: E402
from concourse.bass_utils import run_bass_kernel_spmd  # noqa: E402
from concourse.library_config import mlp             # noqa: E402

bf16 = ml_dtypes.bfloat16
f16 = np.float16
dt = mybir.dt
AF = mybir.ActivationFunctionType

N, IN_DIM, HID, HEADS, OUT_DIM, E = 50000, 128, 64, 2, 64, 1600000
NCORES = 8
NPC = N // NCORES            # 6250
WIN = 127                    # dst nodes per window (col 127 = pad trash)
NWIN = -(-NPC // WIN)        # 50
TILE = 128
HALF_LIM = 25024             # src < HALF_LIM -> half A (idx base 0)
BASE_B = HALF_LIM            # half B idx = src - BASE_B in [0, 25023]
NTAB = 391 * 128             # 50048 table rows (N padded to 128)
NSH = 50 * 128               # 6400 shard rows (>= NWIN*WIN = 6350)
OUT_ROWS = NWIN * WIN        # 6350

# module-level memo: preprocessing + compiled programs are reused across calls
_CACHE = {}
LAST_EXEC_NS = []            # exec_time_ns of the launches from the last call
LAST_RESULTS = []            # full BassKernelResults of the last call (trace mode)


def _register_ntff_hook():
    """Provide antenv.axon_hooks (absent in this container) so
    run_bass_kernel_spmd(trace=True) can capture NTFF profiles."""
    import types
    import ctypes
    import contextlib

    if "antenv.axon_hooks" in sys.modules:
        return
    try:
        lib = ctypes.CDLL("/opt/axon/libaxon_pjrt.so")
        lib.axon_start_nrt_profile.argtypes = [
            ctypes.POINTER(ctypes.c_int64), ctypes.c_size_t]
        lib.axon_start_nrt_profile.restype = ctypes.c_int64
        lib.axon_stop_nrt_profile.argtypes = [ctypes.c_char_p]
        lib.axon_stop_nrt_profile.restype = ctypes.c_int64
    except (OSError, AttributeError):
        return

    @contextlib.contextmanager
    def _hook(output_dir, device_ids):
        import jax
        jax.devices()
        if device_ids:
            ids = (ctypes.c_int64 * len(device_ids))(*device_ids)
            rc = lib.axon_start_nrt_profile(ids, len(device_ids))
        else:
            rc = lib.axon_start_nrt_profile(None, 0)
        if rc != 0:
            raise RuntimeError(f"axon_start_nrt_profile rc={rc}")
        try:
            yield
        finally:
            n = lib.axon_stop_nrt_profile(str(output_dir).encode())
            print(f"ntff profile: {n} file(s) -> {output_dir}", file=sys.stderr)

    mod = types.ModuleType("antenv.axon_hooks")
    mod.get_axon_ntff_profile_hook = lambda: _hook
    sys.modules["antenv.axon_hooks"] = mod
    # avoid network uploads during offline trace processing
    import concourse.bass_utils as _bu
    _bu.upload_artifacts = lambda p: str(p)


# --------------------------------------------------------------------------
# host-side graph preprocessing (index-only)
# --------------------------------------------------------------------------

def _schedule(edge_index):
    src = np.concatenate([edge_index[0], np.arange(N)]).astype(np.int64)
    dst = np.concatenate([edge_index[1], np.arange(N)]).astype(np.int64)
    shard = dst // NPC

    # collect per (core, window, half) edge lists
    per = [[None] * NWIN for _ in range(NCORES)]
    for c in range(NCORES):
        m = shard == c
        s, d = src[m], dst[m] - c * NPC
        wi = d // WIN
        for w in range(NWIN):
            wm = wi == w
            ws, wd = s[wm], d[wm] - w * WIN
            a = ws < HALF_LIM
            per[c][w] = ((ws[a], wd[a]), (ws[~a], wd[~a]))

    # uniform tile counts per (window, half) = max over cores
    nA = [max(-(-len(per[c][w][0][0]) // TILE) for c in range(NCORES))
          for w in range(NWIN)]
    nB = [max(-(-len(per[c][w][1][0]) // TILE) for c in range(NCORES))
          for w in range(NWIN)]
    ntot = sum(nA) + sum(nB)

    t1 = np.zeros((NCORES, ntot * TILE), np.int16)
    dr = np.zeros((NCORES, ntot * TILE), np.float32)
    for c in range(NCORES):
        pos = 0
        for w in range(NWIN):
            for half, ntiles in ((0, nA[w]), (1, nB[w])):
                ws, wd = per[c][w][half]
                ne, cap = len(ws), ntiles * TILE
                pad = cap - ne
                base = 0 if half == 0 else BASE_B
                # pad idx = -1: the Q7 ucode trims trailing negative indices,
                # so each core only generates descriptors for its true edges
                if os.environ.get("K_TRIM", "0") == "1":
                    fs = np.concatenate([ws - base, np.full(pad, -1)])
                else:
                    psrc = np.full(pad, ws[-1] if ne else base)
                    fs = np.concatenate([ws, psrc]) - base
                fd = np.concatenate([wd, np.full(pad, WIN)])
                t1[c, pos:pos + cap] = fs.astype(np.int16)
                dr[c, pos:pos + cap] = fd
                pos += cap
        assert pos == ntot * TILE

    def wrap_idx(a):  # -> [128, n/16] wrapped for the 8 Q7 cores
        return np.ascontiguousarray(np.tile(a.reshape(-1, 16).T, (8, 1)))

    i1 = [wrap_idx(t1[c]) for c in range(NCORES)]
    drel = [np.ascontiguousarray(dr[c].reshape(-1, TILE).T.astype(bf16))
            for c in range(NCORES)]
    drel_rm = [np.ascontiguousarray(dr[c].reshape(-1, TILE).astype(f16))
               for c in range(NCORES)]
    return {"nA": nA, "nB": nB, "ntot": ntot, "i1": i1, "drel": drel,
            "drel_rm": drel_rm}


def _expand_att(a):
    """att [heads, dim] -> [128, heads] block-diagonal expansion (layout only)."""
    heads, dim = a.shape
    out = np.zeros((heads * dim, heads), np.float32)
    for h in range(heads):
        out[h * dim:(h + 1) * dim, h] = a[h]
    return out.astype(f16)


# --------------------------------------------------------------------------
# device program (identical for all cores; layer 1/2 differ only in flush)
# --------------------------------------------------------------------------

def _build_program(layer, sched, nwin=NWIN):
    nA, nB, ntot = sched["nA"], sched["nB"], sched["ntot"]
    GW = max(nA[w] + nB[w] for w in range(nwin))
    NSWQ = int(os.environ.get("K_NSWQ", "2"))
    nc = bacc.Bacc("TRN2", target_bir_lowering=False, debug=False,
                   enable_asserts=False, num_devices=NCORES,
                   num_swdge_queues=NSWQ)

    xT = nc.dram_tensor("xT", [128, NTAB], dt.float16, kind="ExternalInput")
    xTs = nc.dram_tensor("xTs", [128, NSH], dt.float16, kind="ExternalInput")
    W = nc.dram_tensor("W", [128, 128], dt.float16, kind="ExternalInput")
    WT = nc.dram_tensor("WT", [128, 128], dt.float16, kind="ExternalInput")
    Ase = nc.dram_tensor("Ase", [128, 2], dt.float16, kind="ExternalInput")
    Ade = nc.dram_tensor("Ade", [128, 2], dt.float16, kind="ExternalInput")
    brep = nc.dram_tensor("brep", [128, 128], dt.float32, kind="ExternalInput")
    iota = nc.dram_tensor("iota", [128, 128], dt.bfloat16, kind="ExternalInput")
    iotaP = nc.dram_tensor("iotaP", [128, 1], dt.float16, kind="ExternalInput")
    i1d = nc.dram_tensor("i1", [128, ntot * 8], dt.int16, kind="ExternalInput")
    dreld = nc.dram_tensor("drel", [128, ntot], dt.bfloat16, kind="ExternalInput")
    drelrmd = nc.dram_tensor("drel_rm", [ntot, TILE], dt.float16,
                             kind="ExternalInput")
    if layer == 1:
        outd = nc.dram_tensor("out", [OUT_ROWS, 128], dt.float16,
                              kind="ExternalOutput")
    else:
        outd = nc.dram_tensor("out", [OUT_ROWS, 64], dt.float32,
                              kind="ExternalOutput")

    with tile.TileContext(nc) as tc:
        with (
            tc.tile_pool(name="const", bufs=1) as constp,
            tc.tile_pool(name="tb", bufs=3) as tbp,
            tc.tile_pool(name="work", bufs=2) as work,
            tc.tile_pool(name="adp", bufs=3) as adp,
            tc.tile_pool(name="fl", bufs=2) as flp,
            tc.tile_pool(name="pst", bufs=2, space="PSUM") as pst,
            tc.tile_pool(name="psw", bufs=2, space="PSUM") as psw,
            tc.tile_pool(name="psa", bufs=2, space="PSUM") as psa,
            tc.tile_pool(name="dram", bufs=1, space="DRAM") as dram,
        ):
            nc.gpsimd.load_library(mlp)

            T1_dram = dram.tile([NTAB, 256], dt.float16)
            T2_dram = dram.tile([NSH, 128], dt.float16)

            # ---- constants
            iota_sb = constp.tile([128, 128], dt.bfloat16)
            nc.sync.dma_start(iota_sb[:], iota[:])
            iotap_sb = constp.tile([128, 1], dt.float16)
            nc.sync.dma_start(iotap_sb[:], iotaP[:])
            brep_sb = constp.tile([128, 128], dt.float32)
            nc.sync.dma_start(brep_sb[:], brep[:])
            i1_sb = constp.tile([128, ntot * 8], dt.int16)
            nc.sync.dma_start(i1_sb[:], i1d[:])
            drel_sb = constp.tile([128, ntot], dt.bfloat16)
            nc.sync.dma_start(drel_sb[:], dreld[:])

            # ---- weight fold: We = [W | W @ Ase]; wd = W @ Ade
            wt_sb = constp.tile([128, 128], dt.float16)
            nc.sync.dma_start(wt_sb[:], WT[:])
            ase_sb = constp.tile([128, 2], dt.float16)
            nc.sync.dma_start(ase_sb[:], Ase[:])
            ade_sb = constp.tile([128, 2], dt.float16)
            nc.sync.dma_start(ade_sb[:], Ade[:])
            we_sb = constp.tile([128, 130], dt.float16)
            nc.sync.dma_start(we_sb[:, 0:128], W[:])
            wd_sb = constp.tile([128, 2], dt.float16)
            ps = pst.tile([128, 2], dt.float32, tag="pt")
            nc.tensor.matmul(ps[:], wt_sb[:], ase_sb[:])
            nc.scalar.activation(out=we_sb[:, 128:130], in_=ps[:], func=AF.Copy)
            ps2 = pst.tile([128, 2], dt.float32, tag="pt")
            nc.tensor.matmul(ps2[:], wt_sb[:], ade_sb[:])
            nc.scalar.activation(out=wd_sb[:], in_=ps2[:], func=AF.Copy)

            # ---- T2 table build first (windows' AdRep depends on it)
            for i in range(NSH // 128):
                xs = tbp.tile([128, 128], dt.float16, tag="xs")
                nc.sync.dma_start(xs[:], xTs[:, i * 128:(i + 1) * 128])
                p2 = pst.tile([128, 2], dt.float32, tag="pt2")
                nc.tensor.matmul(p2[:], xs[:], wd_sb[:])
                t2b = tbp.tile([128, 2], dt.float16, tag="t2out")
                nc.scalar.activation(out=t2b[:], in_=p2[:], func=AF.Copy)
                nc.sync.dma_start(T2_dram[i * 128:(i + 1) * 128, 0:2], t2b[:])

            # ---- T1 table build: [h | alpha_src] for all N.
            # DMAs batched 8 iterations at a time: the per-dma_start issue
            # cost on the sync sequencer (~0.7us) dominated the build.
            USE_BATCHED_BUILD = os.environ.get("K_BATCH", "0") == "1"
            if USE_BATCHED_BUILD:
                B1 = 8
                for j0 in range(0, NTAB // 128, B1):
                    nb = min(B1, NTAB // 128 - j0)
                    xt = tbp.tile([128, B1 * 128], dt.float16, tag="xt")
                    nc.sync.dma_start(xt[:, 0:nb * 128],
                                      xT[:, j0 * 128:(j0 + nb) * 128])
                    tb = tbp.tile([128, B1, 130], dt.float16, tag="tbout")
                    for k in range(nb):
                        pt = pst.tile([128, 130], dt.float32, tag="pt")
                        nc.tensor.matmul(pt[:], xt[:, k * 128:(k + 1) * 128],
                                         we_sb[:])
                        nc.scalar.activation(out=tb[:, k, :], in_=pt[:],
                                             func=AF.Copy)
                    dview = T1_dram[j0 * 128:(j0 + nb) * 128, 0:130]
                    dap = bass.AP(tensor=dview.tensor, offset=dview.offset,
                                  ap=[[256, 128], [128 * 256, nb], [1, 130]])
                    nc.sync.dma_start(dap, tb[:, 0:nb, :])
            else:
                for i in range(NTAB // 128):
                    xt = tbp.tile([128, 128], dt.float16, tag="xt")
                    nc.sync.dma_start(xt[:], xT[:, i * 128:(i + 1) * 128])
                    pt = pst.tile([128, 130], dt.float32, tag="pt")
                    nc.tensor.matmul(pt[:], xt[:], we_sb[:])
                    tb = tbp.tile([128, 130], dt.float16, tag="tbout")
                    nc.scalar.activation(out=tb[:], in_=pt[:], func=AF.Copy)
                    nc.sync.dma_start(T1_dram[i * 128:(i + 1) * 128, 0:130],
                                      tb[:])

            # ---- edge pipeline
            t1A = T1_dram[0:HALF_LIM, :]
            t1B = T1_dram[BASE_B:NTAB, :]
            def flush_window(w, pw):
                # denom >= exp(LR(-16)) ~ 0.04 for real rows (self loop);
                # +1e-6 keeps the trash/pad rows away from reciprocal(0).
                rd = flp.tile([128, 2], dt.float32, tag="rd")
                nc.vector.tensor_scalar(
                    out=rd[:], in0=pw[:, 128:130], scalar1=1e-6, scalar2=None,
                    op0=AluOpType.add)
                r = flp.tile([128, 2], dt.float32, tag="r")
                nc.vector.reciprocal(r[:], rd[:])
                if layer == 1:
                    f32t = flp.tile([128, 128], dt.float32, tag="f32")
                    for h in range(HEADS):
                        nc.vector.scalar_tensor_tensor(
                            out=f32t[:, h * 64:(h + 1) * 64],
                            in0=pw[:, h * 64:(h + 1) * 64],
                            scalar=r[:, h:h + 1],
                            in1=brep_sb[:, h * 64:(h + 1) * 64],
                            op0=AluOpType.mult, op1=AluOpType.add)
                    ob = flp.tile([128, 128], dt.float16, tag="ob")
                    nc.scalar.activation(out=ob[:], in_=f32t[:], func=AF.Relu)
                    nc.sync.dma_start(outd[w * WIN:(w + 1) * WIN, :],
                                      ob[0:WIN, :])
                else:
                    ta = flp.tile([128, 64], dt.float32, tag="ta")
                    nc.vector.tensor_scalar(
                        out=ta[:], in0=pw[:, 0:64], scalar1=r[:, 0:1],
                        scalar2=None, op0=AluOpType.mult)
                    tb2 = flp.tile([128, 64], dt.float32, tag="tb2")
                    nc.vector.scalar_tensor_tensor(
                        out=tb2[:], in0=pw[:, 64:128], scalar=r[:, 1:2],
                        in1=ta[:], op0=AluOpType.mult, op1=AluOpType.add)
                    ob2 = flp.tile([128, 64], dt.float32, tag="ob2")
                    nc.vector.scalar_tensor_tensor(
                        out=ob2[:], in0=tb2[:], scalar=0.5,
                        in1=brep_sb[:, 0:64], op0=AluOpType.mult,
                        op1=AluOpType.add)
                    nc.sync.dma_start(outd[w * WIN:(w + 1) * WIN, :],
                                      ob2[0:WIN, :])

            tioff = [0]
            for w in range(nwin):
                tioff.append(tioff[-1] + nA[w] + nB[w])

            def stage_pre(w):
                """Gathers + one-hots + alpha_dst load for window w (all
                independent of other windows' compute)."""
                ti, g = tioff[w], nA[w] + nB[w]
                # alpha_dst of this window, partition-major. 128 rows (not
                # 127): row 127 pairs with the pad one-hot and must be a
                # finite value, not uninitialized SBUF.
                adwin = adp.tile([128, 2], dt.float16, tag="adwin")
                nc.sync.dma_start(adwin[:],
                                  T2_dram[w * WIN:w * WIN + 128, 0:2])
                T1g = work.tile([128, GW, 256], dt.float16, tag="t1g")
                # split each half-gather across SWDGE queues: each queue has
                # its own Q7 cpu pair (parallel descriptor generation) and
                # its own descriptor ring (2048 rows/gather fills a ring).
                subs = []           # (tile_lo, tile_hi, src_view)
                if nA[w]:
                    if NSWQ >= 4 and nA[w] > 1:
                        h1 = (nA[w] + 1) // 2
                        subs += [(0, h1, t1A), (h1, nA[w], t1A)]
                    else:
                        subs += [(0, nA[w], t1A)]
                if nB[w]:
                    if NSWQ >= 4 and nB[w] > 1:
                        h1 = (nB[w] + 1) // 2
                        subs += [(nA[w], nA[w] + h1, t1B),
                                 (nA[w] + h1, g, t1B)]
                    else:
                        subs += [(nA[w], g, t1B)]
                for q, (lo, hi, view) in enumerate(subs):
                    ne = (hi - lo) * TILE
                    nc.gpsimd.dma_gather(
                        T1g[:, lo:hi, :], view,
                        i1_sb[:, (ti + lo) * 8:(ti + hi) * 8], ne, ne, 256,
                        single_packet=False, queue_num=q % NSWQ)

                # one-hot S[e, t, j] = (iota_j == dst_rel)
                Sg = work.tile([128, GW, 128], dt.bfloat16, tag="sg")
                io = iota_sb[:]
                io3 = bass.AP(tensor=io.tensor, offset=io.offset,
                              ap=[io.ap[0], [0, g], [1, 128]])
                drs = drel_sb[:, ti:ti + g]
                dr3 = bass.AP(tensor=drs.tensor, offset=drs.offset,
                              ap=[drs.ap[0], [1, g], [0, 128]])
                nc.vector.tensor_tensor(out=Sg[:, 0:g, :], in0=io3,
                                        in1=dr3, op=AluOpType.is_equal)

                # transposed one-hot SgT[j, t, e] = (j == dst_rel[t, e]),
                # from a partition-replicated row-major dst_rel (0-stride DMA)
                drm = work.tile([128, GW, 128], dt.float16, tag="drm")
                dv = drelrmd[ti:ti + g, :]
                dap0 = bass.AP(tensor=dv.tensor, offset=dv.offset,
                               ap=[[0, 128], [TILE, g], [1, TILE]])
                nc.sync.dma_start(drm[:, 0:g, :], dap0)
                SgT = work.tile([128, GW, 128], dt.float16, tag="sgt")
                ipv = iotap_sb[:]
                ip3 = bass.AP(tensor=ipv.tensor, offset=ipv.offset,
                              ap=[ipv.ap[0], [0, g], [0, 128]])
                nc.vector.tensor_tensor(out=SgT[:, 0:g, :], in0=ip3,
                                        in1=drm[:, 0:g, :],
                                        op=AluOpType.is_equal)
                return (adwin, T1g, Sg, SgT)

            def stage_main(w, pre):
                """admm -> score -> exp -> messages -> scatter matmuls."""
                adwin, T1g, Sg, SgT = pre
                ti, g = tioff[w], nA[w] + nB[w]
                # per-edge alpha_dst via TensorE: ad[e, h] = SgT_t.T @ adwin
                psad = psa.tile([128, GW, 2], dt.float32, tag="psad")
                for t in range(g):
                    nc.tensor.matmul(psad[:, t, :], SgT[:, t, :], adwin[:],
                                     start=True, stop=True)
                adc = work.tile([128, GW, 2], dt.float16, tag="adc")
                nc.scalar.activation(out=adc[:, 0:g, :], in_=psad[:, 0:g, :],
                                     func=AF.Copy)
                # compact copy of alpha_src (strided read is slow on DVE)
                asg = work.tile([128, GW, 2], dt.float16, tag="asg")
                nc.scalar.activation(out=asg[:, 0:g, :],
                                     in_=T1g[:, 0:g, 128:130], func=AF.Copy)

                # score = leaky_relu(as + ad); w = exp(score)
                sc = work.tile([128, GW, 2], dt.float32, tag="sc")
                nc.vector.tensor_tensor(
                    out=sc[:, 0:g, :], in0=asg[:, 0:g, :],
                    in1=adc[:, 0:g, :], op=AluOpType.add)
                nc.vector.scalar_tensor_tensor(
                    out=sc[:, 0:g, :], in0=sc[:, 0:g, :], scalar=0.2,
                    in1=sc[:, 0:g, :], op0=AluOpType.mult,
                    op1=AluOpType.max)
                Mg = work.tile([128, GW, 130], dt.bfloat16, tag="mg")
                nc.scalar.activation(
                    out=Mg[:, 0:g, 128:130], in_=sc[:, 0:g, :],
                    func=AF.Exp)
                wb = Mg[:, 0:g, 128:130]
                win1 = bass.AP(tensor=wb.tensor, offset=wb.offset,
                               ap=[wb.ap[0], [130, g], [1, 2], [0, 64]])
                nc.vector.tensor_tensor(
                    out=Mg[:, 0:g, 0:128].rearrange(
                        "p t (h d) -> p t h d", h=2),
                    in0=T1g[:, 0:g, 0:128].rearrange(
                        "p t (h d) -> p t h d", h=2),
                    in1=win1, op=AluOpType.mult)
                pw = psw.tile([128, 130], dt.float32, tag="pw")
                for t in range(g):
                    nc.tensor.matmul(
                        pw[:], Sg[:, t, :], Mg[:, t, :],
                        start=(t == 0), stop=(t == g - 1))
                return pw

            # 2-stage software pipeline: emit window w+1's gather/one-hot
            # stage before window w's compute stage, and flush one window
            # late, so no engine's in-order queue stalls on another engine.
            pre = stage_pre(0)
            pending = None
            for w in range(nwin):
                nxt = stage_pre(w + 1) if w + 1 < nwin else None
                pw = stage_main(w, pre)
                pre = nxt
                if pending is not None:
                    flush_window(*pending)
                pending = (w, pw)
            flush_window(*pending)

    nc.compile()
    return nc


# --------------------------------------------------------------------------
# host orchestration
# --------------------------------------------------------------------------

def _pad_T(x16, cols):
    """[N, 128] fp16 -> transposed padded [128, cols]."""
    out = np.zeros((128, cols), f16)
    out[:, :x16.shape[0]] = x16.T
    return out


def _layer_inputs(sched, xfullT, xshardTs, Wm, att_s, att_d, bias, layer):
    Wf = Wm.astype(f16)
    base = {
        "xT": xfullT,
        "W": np.ascontiguousarray(Wf),
        "WT": np.ascontiguousarray(Wf.T),
        "Ase": _expand_att(att_s),
        "Ade": _expand_att(att_d),
        "iota": np.broadcast_to(np.arange(128, dtype=np.float32),
                                (128, 128)).astype(bf16).copy(),
        "iotaP": np.arange(128, dtype=np.float32).reshape(128, 1).astype(f16),
    }
    br = np.zeros((128, 128), np.float32)
    if layer == 1:
        br[:, :] = bias[None, :]
    else:
        br[:, 0:64] = bias[None, :]
    base["brep"] = br
    maps = []
    for c in range(NCORES):
        m = dict(base)
        m["xTs"] = xshardTs[c]
        m["i1"] = sched["i1"][c]
        m["drel"] = sched["drel"][c]
        m["drel_rm"] = sched["drel_rm"][c]
        maps.append(m)
    return maps


def kernel(**inputs):
    global LAST_EXEC_NS, LAST_RESULTS
    LAST_EXEC_NS = []
    LAST_RESULTS = []
    x = np.asarray(inputs["x"], np.float32)
    edge_index = np.asarray(inputs["edge_index"]).astype(np.int64)

    key = hash(edge_index.tobytes())
    if key not in _CACHE:
        sched = _schedule(edge_index)
        nc1 = _build_program(1, sched)
        nc2 = _build_program(2, sched)
        _CACHE.clear()
        _CACHE[key] = (sched, nc1, nc2)
    sched, nc1, nc2 = _CACHE[key]

    trace = bool(os.environ.get("KERNEL_TRACE"))
    trace_kwargs = {}
    if trace:
        _register_ntff_hook()

    def run(nc, maps):
        res = run_bass_kernel_spmd(nc, maps, core_ids=list(range(NCORES)),
                                   trace=trace, **trace_kwargs)
        LAST_EXEC_NS.append(res.exec_time_ns)
        LAST_RESULTS.append(res)
        return res.results

    # ---------------- launch 1
    x16 = x.astype(f16)
    xfullT = _pad_T(x16, NTAB)
    xshardTs = [np.ascontiguousarray(
        _pad_T(x16[c * NPC:(c + 1) * NPC], NSH)) for c in range(NCORES)]
    maps1 = _layer_inputs(sched, xfullT, xshardTs,
                          np.asarray(inputs["W1"]),
                          np.asarray(inputs["att_src1"]),
                          np.asarray(inputs["att_dst1"]),
                          np.asarray(inputs["b1"], np.float32), 1)
    res1 = run(nc1, maps1)
    out1 = np.concatenate([res1[c]["out"][:NPC] for c in range(NCORES)], 0)

    # ---------------- launch 2
    o16 = out1.astype(f16)
    ofullT = _pad_T(o16, NTAB)
    oshardTs = [np.ascontiguousarray(
        _pad_T(o16[c * NPC:(c + 1) * NPC], NSH)) for c in range(NCORES)]
    maps2 = _layer_inputs(sched, ofullT, oshardTs,
                          np.asarray(inputs["W2"]),
                          np.asarray(inputs["att_src2"]),
                          np.asarray(inputs["att_dst2"]),
                          np.asarray(inputs["b2"], np.float32), 2)
    res2 = run(nc2, maps2)
    out2 = np.concatenate([res2[c]["out"][:NPC] for c in range(NCORES)], 0)
    return out2.astype(np.float32)
